# revision 58
# baseline (speedup 1.0000x reference)
"""Trainium2 Bass kernel for nn_Attention_3917010174340.

Reference computation (per batch b):
    first_score = hs @ W1 + b1            # [T, H]
    h_t   = first_score[-1]               # [H]
    score = first_score @ h_t             # [T]
    w     = softmax(score)
    ctx   = hs^T @ w                      # [H]
    out   = tanh(concat([ctx, h_t]) @ Wv + bv)

Algebraic rewrite used here (the big matmul collapses):
    score[t] = hs[t] . (W1 @ h_t) + (b1 . h_t)
The (b1 . h_t) term is constant over t and cancels inside softmax, so with
v = W1 @ h_t:
    w = softmax(hs @ v)
This turns a 137 GFLOP matmul problem into an HBM-bound streaming problem
(read hs once) plus tiny matvecs.

Sharding: data-parallel over batch B=32 across 8 cores (4 batches/core),
weights replicated. Each core streams its 32 MiB hs shard once (SP HWDGE
ring), weights go on the gpsimd/ACT rings so they never head-of-line block
the hs stream. Scores are fused multiply-reduce on DVE; the context weighted
sum runs on PE with the softmax weights as the stationary operand and hs
streaming in float32r mode; batches are software-pipelined (score of batch
b+1 is emitted before softmax/context of batch b).
"""

import numpy as np

import concourse.bacc as bacc
import concourse.bass as bass
import concourse.tile as tile
from concourse import mybir
from concourse.bass_utils import run_bass_kernel_spmd


F32 = mybir.dt.float32
F32R = mybir.dt.float32r
ALU = mybir.AluOpType
ACTF = mybir.ActivationFunctionType

B, T, H, OUT = 32, 2048, 1024, 256
NCORES = 8
BL = B // NCORES          # batches per core = 4
P = 128                   # partitions
HC = H // P               # 8 h-chunks
TC = T // P               # 16 t-chunks per batch
HTC = TC // 2             # 8 t-chunks per half-batch


def _build_body(ctx, nc, tc, hs, w1, w1t, b1, wv, bv, cin, y):
    # ---------------- pools ----------------
    consts = ctx.enter_context(tc.tile_pool(name="consts", bufs=1))
    small = ctx.enter_context(tc.tile_pool(name="small", bufs=1))
    w1pool = ctx.enter_context(tc.tile_pool(name="w1", bufs=3))
    vbpool = ctx.enter_context(tc.tile_pool(name="vb", bufs=1))
    hspool = ctx.enter_context(tc.tile_pool(name="hsp", bufs=4))
    wvpool = ctx.enter_context(tc.tile_pool(name="wvp", bufs=1))
    sb_batch = ctx.enter_context(tc.tile_pool(name="sbb", bufs=2))

    ps_big = ctx.enter_context(tc.tile_pool(name="psbig", bufs=1, space="PSUM"))
    ps_tr = ctx.enter_context(tc.tile_pool(name="pstr", bufs=5, space="PSUM"))
    ps_fin = ctx.enter_context(tc.tile_pool(name="psfin", bufs=1, space="PSUM"))

    # ---------------- constants (SP ring: tiny, ahead of the hs stream) ----
    identity = consts.tile([P, P], F32)
    nc.sync.dma_start(out=identity, in_=cin)
    ones_col = consts.tile([P, 1], F32)
    nc.vector.memset(ones_col, 1.0)
    neg_ones_row = consts.tile([1, P], F32)
    nc.vector.memset(neg_ones_row, -1.0)

    def bcast_rows(src, n):
        return bass.AP(tensor=src.tensor, offset=src.offset, ap=[[0, n]] + list(src.ap))

    bv_b4 = consts.tile([BL, OUT], F32)
    nc.sync.dma_start(out=bv_b4, in_=bcast_rows(bv, BL))

    # internal DRAM scratch for the v broadcast roundtrip
    vdram = nc.dram_tensor("v_scratch", [BL, H], F32).ap()

    # last timestep of hs: load naturally, PE-transpose into [p, jc, b]
    # (all 8 transposes land in one PSUM tile -> single copy out)
    hs_last_sb = w1pool.tile([BL, H], F32, tag="w1c", bufs=2)
    nc.sync.dma_start(out=hs_last_sb, in_=hs[:, T - 1, :])
    hs_lastT = consts.tile([P, HC, BL], F32)
    ps_lt = ps_tr.tile([P, HC, BL], F32, tag="tr")
    for jc in range(HC):
        nc.tensor.transpose(
            ps_lt[:, jc, :], hs_last_sb[:, jc * P : (jc + 1) * P], identity[:BL, :BL]
        )
    nc.vector.tensor_copy(out=hs_lastT, in_=ps_lt)

    # ---------------- phase 0: h_t, v, v_bcast ----------------
    # W1T super-chunks stream on the ACT HWDGE ring (f32r for the v matmuls),
    # W1 super-chunks on the gpsimd SWDGE ring — neither touches the SP ring,
    # which is reserved for the hs stream.
    w1t_chunks = []
    for sc in range(4):
        w1tc = w1pool.tile([P, 2, H], F32R, tag="w1tc", bufs=2)
        nc.scalar.dma_start(
            out=w1tc,
            in_=w1t[sc * 256 : (sc + 1) * 256, :]
            .rearrange("(c p) h -> p c h", p=P)
            .bitcast(F32R),
        )
        w1t_chunks.append(w1tc)
    # Wv (f32r) also on the ACT ring, behind W1T
    wv_sb = wvpool.tile([P, 2 * HC, OUT], F32R)
    nc.scalar.dma_start(
        out=wv_sb, in_=wv.rearrange("(c p) o -> p c o", p=P).bitcast(F32R)
    )

    # h_t[b, k] = sum_j hs_last[b, j] * W1[j, k]  (+ b1), full fp32
    ps_ht = ps_big.tile([BL, H], F32, tag="big")
    for sc in range(4):
        w1c = w1pool.tile([P, 2, H], F32, tag="w1c", bufs=2)
        nc.gpsimd.dma_start(
            out=w1c,
            in_=w1[sc * 256 : (sc + 1) * 256, :].rearrange("(c p) h -> p c h", p=P),
        )
        for jj in range(2):
            jc = sc * 2 + jj
            for nn in range(2):
                nc.tensor.matmul(
                    ps_ht[:, nn * 512 : (nn + 1) * 512],
                    lhsT=hs_lastT[:, jc, :],
                    rhs=w1c[:, jj, nn * 512 : (nn + 1) * 512],
                    start=(jc == 0),
                    stop=(jc == HC - 1),
                )
    # b1 broadcast rides a freed w1c slot (emitted after the W1 loop so its
    # slot-WAR wait doesn't head-of-line block the W1 loads on the gpsimd ring)
    b1_b4 = w1pool.tile([BL, H], F32, tag="w1c", bufs=2)
    nc.gpsimd.dma_start(out=b1_b4, in_=bcast_rows(b1, BL))
    ht_sb = small.tile([BL, H], F32, tag="htv", bufs=1)
    nc.vector.tensor_copy(out=ht_sb, in_=ps_ht)
    nc.vector.tensor_add(ht_sb, ht_sb, b1_b4)

    # h_t transposed into [p, kc, b]; stored f32r (DVE rounding copy) for the
    # f32r v / final-projection matmuls
    htT = consts.tile([P, HC, BL], F32R)
    ps_htt = ps_tr.tile([P, HC, BL], F32, tag="tr")
    for kc in range(HC):
        nc.tensor.transpose(
            ps_htt[:, kc, :], ht_sb[:, kc * P : (kc + 1) * P], identity[:BL, :BL]
        )
    nc.vector.tensor_copy(out=htT, in_=ps_htt)

    # v[b, j] = sum_k W1T[k, j] * h_t[b, k]  (f32r: 1 col/cycle)
    ps_v = ps_big.tile([BL, H], F32, tag="big")
    for kc in range(HC):
        w1tc = w1t_chunks[kc // 2]
        kk = kc % 2
        for nn in range(2):
            nc.tensor.matmul(
                ps_v[:, nn * 512 : (nn + 1) * 512],
                lhsT=htT[:, kc, :],
                rhs=w1tc[:, kk, nn * 512 : (nn + 1) * 512],
                start=(kc == 0),
                stop=(kc == HC - 1),
            )
    v_sb = small.tile([BL, H], F32, tag="htv", bufs=1)
    nc.vector.tensor_copy(out=v_sb, in_=ps_v)

    # broadcast v across partitions via DRAM roundtrip (one partition-step-0
    # DMA covering all batches; on the ACT HWDGE ring: low fixed cost, and the
    # ring is idle after the weight loads)
    nc.scalar.dma_start(out=vdram, in_=v_sb)
    v_bcast = vbpool.tile([P, BL, H], F32)
    nc.scalar.dma_start(out=v_bcast, in_=bcast_rows(vdram, P))

    # h_t half of the final projection: accumulate into ps_ab now (the group
    # stays open until the ctx half completes at the end of the kernel)
    ps_ab = ps_fin.tile([BL, OUT], F32)
    for c in range(HC):
        nc.tensor.matmul(
            ps_ab, lhsT=htT[:, c, :], rhs=wv_sb[:, HC + c, :],
            start=(c == 0), stop=False,
        )

    # ---------------- software-pipelined batch loop ----------------
    ctxT = consts.tile([P, HC, BL], F32R)    # normalized context^T (f32r)
    junk = w1pool.tile([P, H], F32, tag="w1c", bufs=2)

    def emit_load(b, gate=None):
        halves = []
        for half in range(2):
            ht_tile = hspool.tile([P, HTC, H], F32R, tag="hs")
            if gate is not None:
                # tiny write that depends on `gate` — orders this slot's DMA
                # after phase 0 so it doesn't steal early HBM bandwidth
                nc.vector.tensor_copy(out=ht_tile[0:1, 0, 0:1], in_=gate[0:1, 0:1])
            nc.sync.dma_start(
                out=ht_tile,
                in_=hs[b, half * (T // 2) : (half + 1) * (T // 2), :]
                .rearrange("(c p) h -> p c h", p=P)
                .bitcast(F32R),
            )
            halves.append(ht_tile)
        return halves

    def emit_score(b, halves):
        score_sb = sb_batch.tile([P, TC], F32, tag="score")
        for half in range(2):
            for c in range(HTC):
                i = half * HTC + c
                nc.vector.scalar_tensor_tensor(
                    out=junk,
                    in0=halves[half][:, c, :].bitcast(F32),
                    scalar=1.0,
                    in1=v_bcast[:, b, :],
                    op0=ALU.mult,
                    op1=ALU.mult,
                    accum_out=score_sb[:, i : i + 1],
                )
        return score_sb

    def emit_softmax(b, halves, score_sb):
        # softmax: global max -> exp (DVE ops here are tiny; emitted before the
        # next batch's score burst so they slot in ahead of it)
        pmax = sb_batch.tile([P, 1], F32, tag="pmax")
        nc.vector.reduce_max(pmax, score_sb, axis=mybir.AxisListType.X)
        ptr_m = ps_tr.tile([1, P], F32, tag="tr")
        nc.tensor.transpose(ptr_m, pmax, identity)
        gmax = sb_batch.tile([1, 1], F32, tag="gmax")
        nc.vector.reduce_max(gmax, ptr_m, axis=mybir.AxisListType.X)
        ptr_nb = ps_tr.tile([P, 1], F32, tag="tr")
        nc.tensor.matmul(ptr_nb, lhsT=neg_ones_row, rhs=gmax, start=True, stop=True)
        negm_bc = sb_batch.tile([P, 1], F32, tag="negmbc")
        nc.scalar.copy(out=negm_bc, in_=ptr_nb)

        w_sb = sb_batch.tile([P, TC], F32R, tag="wexp")
        dexp = sb_batch.tile([P, 1], F32, tag="dexp")
        nc.scalar.activation(
            out=w_sb, in_=score_sb, func=ACTF.Exp, bias=negm_bc, scale=1.0,
            accum_out=dexp,
        )
        ps_den = ps_tr.tile([1, 1], F32, tag="tr")
        nc.tensor.matmul(ps_den, lhsT=dexp, rhs=ones_col, start=True, stop=True)
        inv_b = sb_batch.tile([1, 1], F32, tag="invb")
        nc.vector.reciprocal(inv_b, ps_den)
        return w_sb, inv_b

    def emit_ctx(b, halves, w_sb, inv_b):
        # context: U[j] = sum_t w[t] * hs[t, j] (row form; w stationary,
        # hs moving in float32r at 1 col/cycle)
        ps_u = ps_big.tile([1, H], F32, tag="big")
        for half in range(2):
            for c in range(HTC):
                i = half * HTC + c
                for nn in range(2):
                    nc.tensor.matmul(
                        ps_u[:, nn * 512 : (nn + 1) * 512],
                        lhsT=w_sb[:, i : i + 1],
                        rhs=halves[half][:, c, nn * 512 : (nn + 1) * 512],
                        start=(i == 0),
                        stop=(i == TC - 1),
                    )
        # normalize while copying out of PSUM: ctx_row = U / denom
        ctx_row = sb_batch.tile([1, H], F32, tag="ctxrow", bufs=1)
        nc.scalar.activation(out=ctx_row, in_=ps_u, func=ACTF.Copy, scale=inv_b)
        # transpose into ctxT[:, :, b] (single PSUM tile, one copy on ScalarE
        # to keep DVE free for the score stream)
        ps_ct = ps_tr.tile([P, HC, 1], F32, tag="tr")
        for c in range(HC):
            nc.tensor.transpose(
                ps_ct[:, c, :], ctx_row[0:1, c * P : (c + 1) * P], identity[0:1, 0:1]
            )
        nc.scalar.copy(out=ctxT[:, :, b : b + 1], in_=ps_ct)

    gates = {0: ht_sb, 1: v_sb}
    prev = None
    for b in range(BL):
        halves = emit_load(b, gate=gates.get(b))
        score_sb = emit_score(b, halves)
        if prev is not None:
            emit_ctx(*prev)
        sm = emit_softmax(b, halves, score_sb)
        prev = (b, halves, sm[0], sm[1])
    emit_ctx(*prev)

    # ---------------- final projection (ctx half; h_t half ran in phase 0) --
    for c in range(HC):
        nc.tensor.matmul(
            ps_ab, lhsT=ctxT[:, c, :], rhs=wv_sb[:, c, :],
            start=False, stop=(c == HC - 1),
        )

    res = small.tile([BL, OUT], F32)
    nc.vector.tensor_add(res, ps_ab, bv_b4)
    out_sb = small.tile([BL, OUT], F32)
    nc.scalar.activation(out=out_sb, in_=res, func=ACTF.Tanh)
    nc.sync.dma_start(out=y, in_=out_sb)


def build_program():
    nc = bacc.Bacc("TRN2", target_bir_lowering=False, debug=False)
    hs = nc.dram_tensor("hs", [BL, T, H], F32, kind="ExternalInput").ap()
    w1 = nc.dram_tensor("W1", [H, H], F32, kind="ExternalInput").ap()
    w1t = nc.dram_tensor("W1T", [H, H], F32, kind="ExternalInput").ap()
    b1 = nc.dram_tensor("b1", [H], F32, kind="ExternalInput").ap()
    wv = nc.dram_tensor("Wv", [2 * H, OUT], F32, kind="ExternalInput").ap()
    bv = nc.dram_tensor("bv", [OUT], F32, kind="ExternalInput").ap()
    cin = nc.dram_tensor("cin", [P, P], F32, kind="ExternalInput").ap()
    y = nc.dram_tensor("y", [BL, OUT], F32, kind="ExternalOutput").ap()

    from contextlib import ExitStack

    with tile.TileContext(nc) as tc:
        with ExitStack() as ctx:
            _build_body(ctx, nc, tc, hs, w1, w1t, b1, wv, bv, cin, y)
    nc.compile()
    return nc


_NC_CACHE = None


def _get_program():
    global _NC_CACHE
    if _NC_CACHE is None:
        _NC_CACHE = build_program()
    return _NC_CACHE


def make_cin():
    return np.eye(P, dtype=np.float32)


def kernel(hidden_states, W1, b1, Wv, bv, _trace=False, _trace_kwargs=None):
    hs = np.ascontiguousarray(np.asarray(hidden_states, dtype=np.float32))
    W1 = np.ascontiguousarray(np.asarray(W1, dtype=np.float32))
    b1 = np.ascontiguousarray(np.asarray(b1, dtype=np.float32))
    Wv = np.ascontiguousarray(np.asarray(Wv, dtype=np.float32))
    bv = np.ascontiguousarray(np.asarray(bv, dtype=np.float32))
    W1T = np.ascontiguousarray(W1.T)

    nc = _get_program()
    cin = make_cin()
    in_maps = [
        {
            "hs": hs[i * BL : (i + 1) * BL],
            "W1": W1,
            "W1T": W1T,
            "b1": b1,
            "Wv": Wv,
            "bv": bv,
            "cin": cin,
        }
        for i in range(NCORES)
    ]
    kwargs = {}
    if _trace:
        kwargs["trace"] = True
        kwargs.update(_trace_kwargs or {})
    res = run_bass_kernel_spmd(nc, in_maps, core_ids=list(range(NCORES)), **kwargs)
    out = np.concatenate([res.results[i]["y"] for i in range(NCORES)], axis=0)
    if _trace:
        return out, res
    return out


if __name__ == "__main__":
    rng = np.random.default_rng(0)
    ins = {
        "hidden_states": rng.standard_normal((B, T, H), dtype=np.float32),
        "W1": (rng.standard_normal((H, H), dtype=np.float32) / np.sqrt(H)).astype(
            np.float32
        ),
        "b1": np.zeros(H, np.float32),
        "Wv": (
            rng.standard_normal((2 * H, OUT), dtype=np.float32) / np.sqrt(2 * H)
        ).astype(np.float32),
        "bv": np.zeros(OUT, np.float32),
    }
    out = kernel(**ins)
    print("out", out.shape, out.dtype, float(np.abs(out).mean()))


# revision 69
# speedup vs baseline: 1.0853x; 1.0853x over previous
"""Trainium2 Bass kernel for nn_Attention_3917010174340.

Reference computation (per batch b):
    first_score = hs @ W1 + b1            # [T, H]
    h_t   = first_score[-1]               # [H]
    score = first_score @ h_t             # [T]
    w     = softmax(score)
    ctx   = hs^T @ w                      # [H]
    out   = tanh(concat([ctx, h_t]) @ Wv + bv)

Algebraic rewrite used here (the big matmul collapses):
    score[t] = hs[t] . (W1 @ h_t) + (b1 . h_t)
The (b1 . h_t) term is constant over t and cancels inside softmax, so with
v = W1 @ h_t:
    w = softmax(hs @ v)
This turns a 137 GFLOP matmul problem into an HBM-bound streaming problem
(read hs once) plus tiny matvecs.

Sharding: data-parallel over batch B=32 across 8 cores (4 batches/core),
weights replicated. Each core streams its 32 MiB hs shard once (SP HWDGE
ring), weights go on the gpsimd/ACT rings so they never head-of-line block
the hs stream. Scores are fused multiply-reduce on DVE; the context weighted
sum runs on PE with the softmax weights as the stationary operand and hs
streaming in float32r mode; batches are software-pipelined (score of batch
b+1 is emitted before softmax/context of batch b).
"""

import numpy as np

import concourse.bacc as bacc
import concourse.bass as bass
import concourse.tile as tile
from concourse import mybir
from concourse.bass_utils import run_bass_kernel_spmd


F32 = mybir.dt.float32
F32R = mybir.dt.float32r
ALU = mybir.AluOpType
ACTF = mybir.ActivationFunctionType

B, T, H, OUT = 32, 2048, 1024, 256
NCORES = 8
BL = B // NCORES          # batches per core = 4
P = 128                   # partitions
HC = H // P               # 8 h-chunks
TC = T // P               # 16 t-chunks per batch
HTC = TC // 2             # 8 t-chunks per half-batch


def _build_body(ctx, nc, tc, hs, w1, g, c1, b1, wv, bv, cin, y):
    # ---------------- pools ----------------
    consts = ctx.enter_context(tc.tile_pool(name="consts", bufs=1))
    small = ctx.enter_context(tc.tile_pool(name="small", bufs=1))
    w1pool = ctx.enter_context(tc.tile_pool(name="w1", bufs=3))
    vbpool = ctx.enter_context(tc.tile_pool(name="vb", bufs=1))
    hspool = ctx.enter_context(tc.tile_pool(name="hsp", bufs=4))
    wvpool = ctx.enter_context(tc.tile_pool(name="wvp", bufs=1))
    sb_batch = ctx.enter_context(tc.tile_pool(name="sbb", bufs=2))

    ps_big = ctx.enter_context(tc.tile_pool(name="psbig", bufs=1, space="PSUM"))
    ps_tr = ctx.enter_context(tc.tile_pool(name="pstr", bufs=3, space="PSUM"))
    ps_fin = ctx.enter_context(tc.tile_pool(name="psfin", bufs=1, space="PSUM"))

    # ---------------- constants (SP ring: tiny, ahead of the hs stream) ----
    identity = consts.tile([BL, BL], F32)
    nc.sync.dma_start(out=identity, in_=cin[0:BL, 0:BL])
    ones_col = consts.tile([P, 1], F32)
    nc.vector.memset(ones_col, 1.0)
    neg_ones_row = consts.tile([1, P], F32)
    nc.vector.memset(neg_ones_row, -1.0)

    def bcast_rows(src, n):
        return bass.AP(tensor=src.tensor, offset=src.offset, ap=[[0, n]] + list(src.ap))

    bv_b4 = consts.tile([BL, OUT], F32)
    nc.sync.dma_start(out=bv_b4, in_=bcast_rows(bv, BL))

    # internal DRAM scratch for the v broadcast roundtrip
    vdram = nc.dram_tensor("v_scratch", [BL, H], F32).ap()

    # last timestep of hs: load naturally, PE-transpose into [p, jc, b]
    # (all 8 transposes land in one PSUM tile -> single copy out)
    hs_last_sb = w1pool.tile([BL, H], F32, tag="w1c", bufs=2)
    nc.sync.dma_start(out=hs_last_sb, in_=hs[:, T - 1, :])
    hs_lastT = consts.tile([P, HC, BL], F32)
    ps_lt = ps_tr.tile([P, HC, BL], F32, tag="tr")
    for jc in range(HC):
        nc.tensor.transpose(
            ps_lt[:, jc, :], hs_last_sb[:, jc * P : (jc + 1) * P], identity
        )
    nc.vector.tensor_copy(out=hs_lastT, in_=ps_lt)

    # ---------------- phase 0: h_t, v, v_bcast ----------------
    # v = W1 @ h_t collapses via G = W1 @ W1^T (host-precomputed, symmetric):
    #   v = hs_last @ G + (W1 @ b1)
    # so v streams in parallel with h_t straight from hs_lastT — no serial
    # h_t -> transpose -> W1^T chain. G (f32r) on the ACT HWDGE ring, W1 (f32)
    # on the gpsimd SWDGE ring; the SP ring is reserved for the hs stream.
    hs_lastT_r = consts.tile([P, HC, BL], F32R)
    nc.vector.tensor_copy(out=hs_lastT_r, in_=ps_lt)

    g_chunks = []
    for sc in range(4):
        gc_t = w1pool.tile([P, 2, H], F32R, tag="gc", bufs=2)
        nc.scalar.dma_start(
            out=gc_t,
            in_=g[sc * 256 : (sc + 1) * 256, :]
            .rearrange("(c p) h -> p c h", p=P)
            .bitcast(F32R),
        )
        g_chunks.append(gc_t)
    # Wv (f32r) also on the ACT ring, behind G. Split: the h_t half is only
    # needed in phase 0, so it rides a freed G slot; the ctx half persists.
    wv_ctx = wvpool.tile([P, HC, OUT], F32R)
    nc.scalar.dma_start(
        out=wv_ctx, in_=wv[0:H, :].rearrange("(c p) o -> p c o", p=P).bitcast(F32R)
    )
    wv_ht = w1pool.tile([P, HC, OUT], F32R, tag="gc", bufs=2)
    nc.scalar.dma_start(
        out=wv_ht, in_=wv[H : 2 * H, :].rearrange("(c p) o -> p c o", p=P).bitcast(F32R)
    )
    c1_sb = consts.tile([1, H], F32)
    nc.scalar.dma_start(out=c1_sb, in_=c1)
    ones_row4 = consts.tile([1, BL], F32)
    nc.vector.memset(ones_row4, 1.0)

    # v[b, j] = sum_i hs_last[b, i] * G[i, j] + c1[j]   (f32r, 1 col/cycle)
    ps_v = ps_big.tile([BL, H], F32, tag="bigv")
    for ic in range(HC):
        gc_t = g_chunks[ic // 2]
        ii = ic % 2
        for nn in range(2):
            nc.tensor.matmul(
                ps_v[:, nn * 512 : (nn + 1) * 512],
                lhsT=hs_lastT_r[:, ic, :],
                rhs=gc_t[:, ii, nn * 512 : (nn + 1) * 512],
                start=(ic == 0),
                stop=False,
            )
    for nn in range(2):
        nc.tensor.matmul(
            ps_v[:, nn * 512 : (nn + 1) * 512],
            lhsT=ones_row4,
            rhs=c1_sb[:, nn * 512 : (nn + 1) * 512],
            start=False,
            stop=True,
        )
    v_sb = small.tile([BL, H], F32, tag="vsb", bufs=1)
    nc.vector.tensor_copy(out=v_sb, in_=ps_v)

    # h_t[b, k] = sum_j hs_last[b, j] * W1[j, k]  (+ b1), full fp32
    ps_ht = ps_big.tile([BL, H], F32, tag="big")
    for sc in range(4):
        w1c = w1pool.tile([P, 2, H], F32, tag="w1c", bufs=2)
        nc.gpsimd.dma_start(
            out=w1c,
            in_=w1[sc * 256 : (sc + 1) * 256, :].rearrange("(c p) h -> p c h", p=P),
        )
        for jj in range(2):
            jc = sc * 2 + jj
            for nn in range(2):
                nc.tensor.matmul(
                    ps_ht[:, nn * 512 : (nn + 1) * 512],
                    lhsT=hs_lastT[:, jc, :],
                    rhs=w1c[:, jj, nn * 512 : (nn + 1) * 512],
                    start=(jc == 0),
                    stop=(jc == HC - 1),
                )
    # b1 broadcast rides a freed w1c slot (emitted after the W1 loop so its
    # slot-WAR wait doesn't head-of-line block the W1 loads on the gpsimd ring)
    b1_b4 = w1pool.tile([BL, H], F32, tag="w1c", bufs=2)
    nc.gpsimd.dma_start(out=b1_b4, in_=bcast_rows(b1, BL))
    ht_sb = small.tile([BL, H], F32, tag="htv", bufs=1)
    nc.vector.tensor_copy(out=ht_sb, in_=ps_ht)
    nc.vector.tensor_add(ht_sb, ht_sb, b1_b4)

    # h_t transposed into [p, kc, b]; stored f32r (DVE rounding copy) for the
    # final-projection matmuls
    htT = consts.tile([P, HC, BL], F32R)
    ps_htt = ps_tr.tile([P, HC, BL], F32, tag="tr")
    for kc in range(HC):
        nc.tensor.transpose(
            ps_htt[:, kc, :], ht_sb[:, kc * P : (kc + 1) * P], identity
        )
    nc.vector.tensor_copy(out=htT, in_=ps_htt)

    # broadcast v across partitions via DRAM roundtrip (one partition-step-0
    # DMA covering all batches; on the ACT HWDGE ring: low fixed cost, and the
    # ring is idle after the weight loads)
    nc.scalar.dma_start(out=vdram, in_=v_sb)
    v_bcast = vbpool.tile([P, BL, H], F32)
    nc.scalar.dma_start(out=v_bcast, in_=bcast_rows(vdram, P))

    # h_t half of the final projection: accumulate into ps_ab now (the group
    # stays open until the ctx half completes at the end of the kernel)
    ps_ab = ps_fin.tile([BL, OUT], F32)
    for c in range(HC):
        nc.tensor.matmul(
            ps_ab, lhsT=htT[:, c, :], rhs=wv_ht[:, c, :],
            start=(c == 0), stop=False,
        )

    # ---------------- software-pipelined batch loop ----------------
    ctxT = consts.tile([P, HC, BL], F32R)    # normalized context^T (f32r)
    junk = w1pool.tile([P, H], F32, tag="w1c", bufs=2)

    def emit_load(b, gate=None):
        halves = []
        for half in range(2):
            ht_tile = hspool.tile([P, HTC, H], F32R, tag="hs")
            if gate is not None:
                # tiny write that depends on `gate` — orders this slot's DMA
                # after phase 0 so it doesn't steal early HBM bandwidth
                nc.vector.tensor_copy(out=ht_tile[0:1, 0, 0:1], in_=gate[0:1, 0:1])
            nc.sync.dma_start(
                out=ht_tile,
                in_=hs[b, half * (T // 2) : (half + 1) * (T // 2), :]
                .rearrange("(c p) h -> p c h", p=P)
                .bitcast(F32R),
            )
            halves.append(ht_tile)
        return halves

    def emit_score(b, halves):
        score_sb = sb_batch.tile([P, TC], F32, tag="score")
        for half in range(2):
            for c in range(HTC):
                i = half * HTC + c
                nc.vector.scalar_tensor_tensor(
                    out=junk,
                    in0=halves[half][:, c, :].bitcast(F32),
                    scalar=1.0,
                    in1=v_bcast[:, b, :],
                    op0=ALU.mult,
                    op1=ALU.mult,
                    accum_out=score_sb[:, i : i + 1],
                )
        return score_sb

    def emit_softmax(b, halves, score_sb):
        # softmax: global max (one gpsimd all-axis reduce) -> exp
        gmax = sb_batch.tile([1, 1], F32, tag="gmax")
        nc.gpsimd.tensor_reduce(
            gmax, score_sb, axis=mybir.AxisListType.XYZWC, op=ALU.max
        )
        ptr_nb = ps_tr.tile([P, 1], F32, tag="tr")
        nc.tensor.matmul(ptr_nb, lhsT=neg_ones_row, rhs=gmax, start=True, stop=True)
        negm_bc = sb_batch.tile([P, 1], F32, tag="negmbc")
        nc.scalar.copy(out=negm_bc, in_=ptr_nb)

        w_sb = sb_batch.tile([P, TC], F32R, tag="wexp")
        dexp = sb_batch.tile([P, 1], F32, tag="dexp")
        nc.scalar.activation(
            out=w_sb, in_=score_sb, func=ACTF.Exp, bias=negm_bc, scale=1.0,
            accum_out=dexp,
        )
        ps_den = ps_tr.tile([1, 1], F32, tag="tr")
        nc.tensor.matmul(ps_den, lhsT=dexp, rhs=ones_col, start=True, stop=True)
        inv_b = sb_batch.tile([1, 1], F32, tag="invb")
        nc.vector.reciprocal(inv_b, ps_den)
        return w_sb, inv_b

    def emit_ctx(b, halves, w_sb, inv_b):
        # context: U[j] = sum_t w[t] * hs[t, j] (row form; w stationary,
        # hs moving in float32r at 1 col/cycle)
        ps_u = ps_big.tile([1, H], F32, tag="big")
        for half in range(2):
            for c in range(HTC):
                i = half * HTC + c
                for nn in range(2):
                    nc.tensor.matmul(
                        ps_u[:, nn * 512 : (nn + 1) * 512],
                        lhsT=w_sb[:, i : i + 1],
                        rhs=halves[half][:, c, nn * 512 : (nn + 1) * 512],
                        start=(i == 0),
                        stop=(i == TC - 1),
                    )
        # normalize while copying out of PSUM: ctx_row = U / denom
        ctx_row = sb_batch.tile([1, H], F32, tag="ctxrow", bufs=1)
        nc.scalar.activation(out=ctx_row, in_=ps_u, func=ACTF.Copy, scale=inv_b)
        # transpose into ctxT[:, :, b] (single PSUM tile, one copy on ScalarE
        # to keep DVE free for the score stream)
        ps_ct = ps_tr.tile([P, HC, 1], F32, tag="tr")
        for c in range(HC):
            nc.tensor.transpose(
                ps_ct[:, c, :], ctx_row[0:1, c * P : (c + 1) * P], identity[0:1, 0:1]
            )
        nc.scalar.copy(out=ctxT[:, :, b : b + 1], in_=ps_ct)

    gates = {1: v_sb}
    prev = None
    for b in range(BL):
        halves = emit_load(b, gate=gates.get(b))
        score_sb = emit_score(b, halves)
        if prev is not None:
            emit_ctx(*prev)
        sm = emit_softmax(b, halves, score_sb)
        prev = (b, halves, sm[0], sm[1])
    emit_ctx(*prev)

    # ---------------- final projection (ctx half; h_t half ran in phase 0) --
    for c in range(HC):
        nc.tensor.matmul(
            ps_ab, lhsT=ctxT[:, c, :], rhs=wv_ctx[:, c, :],
            start=False, stop=(c == HC - 1),
        )

    res = small.tile([BL, OUT], F32)
    nc.vector.tensor_add(res, ps_ab, bv_b4)
    out_sb = small.tile([BL, OUT], F32)
    nc.scalar.activation(out=out_sb, in_=res, func=ACTF.Tanh)
    nc.sync.dma_start(out=y, in_=out_sb)


def build_program():
    nc = bacc.Bacc("TRN2", target_bir_lowering=False, debug=False)
    hs = nc.dram_tensor("hs", [BL, T, H], F32, kind="ExternalInput").ap()
    w1 = nc.dram_tensor("W1", [H, H], F32, kind="ExternalInput").ap()
    g = nc.dram_tensor("G", [H, H], F32, kind="ExternalInput").ap()
    c1 = nc.dram_tensor("c1", [H], F32, kind="ExternalInput").ap()
    b1 = nc.dram_tensor("b1", [H], F32, kind="ExternalInput").ap()
    wv = nc.dram_tensor("Wv", [2 * H, OUT], F32, kind="ExternalInput").ap()
    bv = nc.dram_tensor("bv", [OUT], F32, kind="ExternalInput").ap()
    cin = nc.dram_tensor("cin", [P, P], F32, kind="ExternalInput").ap()
    y = nc.dram_tensor("y", [BL, OUT], F32, kind="ExternalOutput").ap()

    from contextlib import ExitStack

    with tile.TileContext(nc) as tc:
        with ExitStack() as ctx:
            _build_body(ctx, nc, tc, hs, w1, g, c1, b1, wv, bv, cin, y)
    nc.compile()
    return nc


_NC_CACHE = None


def _get_program():
    global _NC_CACHE
    if _NC_CACHE is None:
        _NC_CACHE = build_program()
    return _NC_CACHE


def make_cin():
    return np.eye(P, dtype=np.float32)


def kernel(hidden_states, W1, b1, Wv, bv, _trace=False, _trace_kwargs=None):
    hs = np.ascontiguousarray(np.asarray(hidden_states, dtype=np.float32))
    W1 = np.ascontiguousarray(np.asarray(W1, dtype=np.float32))
    b1 = np.ascontiguousarray(np.asarray(b1, dtype=np.float32))
    Wv = np.ascontiguousarray(np.asarray(Wv, dtype=np.float32))
    bv = np.ascontiguousarray(np.asarray(bv, dtype=np.float32))
    G = np.ascontiguousarray(W1 @ W1.T)
    c1v = np.ascontiguousarray(W1 @ b1)

    nc = _get_program()
    cin = make_cin()
    in_maps = [
        {
            "hs": hs[i * BL : (i + 1) * BL],
            "W1": W1,
            "G": G,
            "c1": c1v,
            "b1": b1,
            "Wv": Wv,
            "bv": bv,
            "cin": cin,
        }
        for i in range(NCORES)
    ]
    kwargs = {}
    if _trace:
        kwargs["trace"] = True
        kwargs.update(_trace_kwargs or {})
    res = run_bass_kernel_spmd(nc, in_maps, core_ids=list(range(NCORES)), **kwargs)
    out = np.concatenate([res.results[i]["y"] for i in range(NCORES)], axis=0)
    if _trace:
        return out, res
    return out


if __name__ == "__main__":
    rng = np.random.default_rng(0)
    ins = {
        "hidden_states": rng.standard_normal((B, T, H), dtype=np.float32),
        "W1": (rng.standard_normal((H, H), dtype=np.float32) / np.sqrt(H)).astype(
            np.float32
        ),
        "b1": np.zeros(H, np.float32),
        "Wv": (
            rng.standard_normal((2 * H, OUT), dtype=np.float32) / np.sqrt(2 * H)
        ).astype(np.float32),
        "bv": np.zeros(OUT, np.float32),
    }
    out = kernel(**ins)
    print("out", out.shape, out.dtype, float(np.abs(out).mean()))


# revision 73
# speedup vs baseline: 1.4145x; 1.3033x over previous
"""Trainium2 Bass kernel for nn_Attention_3917010174340.

Reference computation (per batch b):
    first_score = hs @ W1 + b1            # [T, H]
    h_t   = first_score[-1]               # [H]
    score = first_score @ h_t             # [T]
    w     = softmax(score)
    ctx   = hs^T @ w                      # [H]
    out   = tanh(concat([ctx, h_t]) @ Wv + bv)

Algebraic rewrite used here (the big matmul collapses):
    score[t] = hs[t] . (W1 @ h_t) + (b1 . h_t)
The (b1 . h_t) term is constant over t and cancels inside softmax, so with
v = W1 @ h_t:
    w = softmax(hs @ v)
This turns a 137 GFLOP matmul problem into an HBM-bound streaming problem
(read hs once) plus tiny matvecs.

Sharding: data-parallel over batch B=32 across 8 cores (4 batches/core),
weights replicated. Each core streams its 32 MiB hs shard once (SP HWDGE
ring), weights go on the gpsimd/ACT rings so they never head-of-line block
the hs stream. Scores are fused multiply-reduce on DVE; the context weighted
sum runs on PE with the softmax weights as the stationary operand and hs
streaming in float32r mode; batches are software-pipelined (score of batch
b+1 is emitted before softmax/context of batch b).
"""

import numpy as np

import concourse.bacc as bacc
import concourse.bass as bass
import concourse.tile as tile
from concourse import mybir
from concourse.bass_utils import run_bass_kernel_spmd


F32 = mybir.dt.float32
F32R = mybir.dt.float32r
ALU = mybir.AluOpType
ACTF = mybir.ActivationFunctionType

B, T, H, OUT = 32, 2048, 1024, 256
NCORES = 8
BL = B // NCORES          # batches per core = 4
P = 128                   # partitions
HC = H // P               # 8 h-chunks
TC = T // P               # 16 t-chunks per batch
HTC = TC // 2             # 8 t-chunks per half-batch


def _build_body(ctx, nc, tc, hs, ht_in, v_in, wv, bv, cin, y):
    # ---------------- pools ----------------
    consts = ctx.enter_context(tc.tile_pool(name="consts", bufs=1))
    small = ctx.enter_context(tc.tile_pool(name="small", bufs=1))
    vbpool = ctx.enter_context(tc.tile_pool(name="vb", bufs=1))
    hspool = ctx.enter_context(tc.tile_pool(name="hsp", bufs=5))
    wvpool = ctx.enter_context(tc.tile_pool(name="wvp", bufs=1))
    sb_batch = ctx.enter_context(tc.tile_pool(name="sbb", bufs=2))

    ps_big = ctx.enter_context(tc.tile_pool(name="psbig", bufs=1, space="PSUM"))
    ps_tr = ctx.enter_context(tc.tile_pool(name="pstr", bufs=3, space="PSUM"))
    ps_fin = ctx.enter_context(tc.tile_pool(name="psfin", bufs=1, space="PSUM"))

    # ---------------- constants (SP ring: tiny, ahead of the hs stream) ----
    identity = consts.tile([BL, BL], F32)
    nc.sync.dma_start(out=identity, in_=cin[0:BL, 0:BL])
    ones_col = consts.tile([P, 1], F32)
    nc.vector.memset(ones_col, 1.0)
    neg_ones_row = consts.tile([1, P], F32)
    nc.vector.memset(neg_ones_row, -1.0)

    def bcast_rows(src, n):
        return bass.AP(tensor=src.tensor, offset=src.offset, ap=[[0, n]] + list(src.ap))

    bv_b4 = consts.tile([BL, OUT], F32)
    nc.sync.dma_start(out=bv_b4, in_=bcast_rows(bv, BL))

    # ---------------- phase 0 (lean): h_t and v come precomputed from host --
    # v_bcast: one partition-step-0 DMA straight from the v input in DRAM,
    # first thing on the ACT HWDGE ring. h_t rides the SP ring (tiny).
    v_bcast = vbpool.tile([P, BL, H], F32)
    nc.scalar.dma_start(out=v_bcast, in_=bcast_rows(v_in, P))
    ht_sb = small.tile([BL, H], F32, tag="htv", bufs=1)
    nc.sync.dma_start(out=ht_sb, in_=ht_in)

    # Wv (f32r) on the ACT ring behind v_bcast; only needed at the end
    wv_ctx = wvpool.tile([P, HC, OUT], F32R)
    nc.scalar.dma_start(
        out=wv_ctx, in_=wv[0:H, :].rearrange("(c p) o -> p c o", p=P).bitcast(F32R)
    )
    wv_ht = wvpool.tile([P, HC, OUT], F32R, tag="wvht")
    nc.scalar.dma_start(
        out=wv_ht, in_=wv[H : 2 * H, :].rearrange("(c p) o -> p c o", p=P).bitcast(F32R)
    )

    # h_t transposed into [p, kc, b]; stored f32r for the final projection
    htT = consts.tile([P, HC, BL], F32R)
    ps_htt = ps_tr.tile([P, HC, BL], F32, tag="tr")
    for kc in range(HC):
        nc.tensor.transpose(
            ps_htt[:, kc, :], ht_sb[:, kc * P : (kc + 1) * P], identity
        )
    nc.vector.tensor_copy(out=htT, in_=ps_htt)

    # h_t half of the final projection: accumulate into ps_ab now (the group
    # stays open until the ctx half completes at the end of the kernel)
    ps_ab = ps_fin.tile([BL, OUT], F32)
    for c in range(HC):
        nc.tensor.matmul(
            ps_ab, lhsT=htT[:, c, :], rhs=wv_ht[:, c, :],
            start=(c == 0), stop=False,
        )

    # ---------------- software-pipelined batch loop ----------------
    ctxT = consts.tile([P, HC, BL], F32R)    # normalized context^T (f32r)
    junk = small.tile([P, H], F32, tag="htv", bufs=1)

    def emit_load(b, gate=None):
        halves = []
        for half in range(2):
            ht_tile = hspool.tile([P, HTC, H], F32R, tag="hs")
            if gate is not None:
                # tiny write that depends on `gate` — orders this slot's DMA
                # after phase 0 so it doesn't steal early HBM bandwidth
                nc.vector.tensor_copy(out=ht_tile[0:1, 0, 0:1], in_=gate[0:1, 0:1])
            nc.sync.dma_start(
                out=ht_tile,
                in_=hs[b, half * (T // 2) : (half + 1) * (T // 2), :]
                .rearrange("(c p) h -> p c h", p=P)
                .bitcast(F32R),
            )
            halves.append(ht_tile)
        return halves

    def emit_score(b, halves):
        score_sb = sb_batch.tile([P, TC], F32, tag="score")
        for half in range(2):
            for c in range(HTC):
                i = half * HTC + c
                nc.vector.scalar_tensor_tensor(
                    out=junk,
                    in0=halves[half][:, c, :].bitcast(F32),
                    scalar=1.0,
                    in1=v_bcast[:, b, :],
                    op0=ALU.mult,
                    op1=ALU.mult,
                    accum_out=score_sb[:, i : i + 1],
                )
        return score_sb

    def emit_softmax(b, halves, score_sb):
        # softmax: global max (one gpsimd all-axis reduce) -> exp
        gmax = sb_batch.tile([1, 1], F32, tag="gmax")
        nc.gpsimd.tensor_reduce(
            gmax, score_sb, axis=mybir.AxisListType.XYZWC, op=ALU.max
        )
        ptr_nb = ps_tr.tile([P, 1], F32, tag="tr")
        nc.tensor.matmul(ptr_nb, lhsT=neg_ones_row, rhs=gmax, start=True, stop=True)
        negm_bc = sb_batch.tile([P, 1], F32, tag="negmbc")
        nc.scalar.copy(out=negm_bc, in_=ptr_nb)

        w_sb = sb_batch.tile([P, TC], F32R, tag="wexp")
        dexp = sb_batch.tile([P, 1], F32, tag="dexp")
        nc.scalar.activation(
            out=w_sb, in_=score_sb, func=ACTF.Exp, bias=negm_bc, scale=1.0,
            accum_out=dexp,
        )
        ps_den = ps_tr.tile([1, 1], F32, tag="tr")
        nc.tensor.matmul(ps_den, lhsT=dexp, rhs=ones_col, start=True, stop=True)
        inv_b = sb_batch.tile([1, 1], F32, tag="invb")
        nc.vector.reciprocal(inv_b, ps_den)
        return w_sb, inv_b

    def emit_ctx(b, halves, w_sb, inv_b):
        # context: U[j] = sum_t w[t] * hs[t, j] (row form; w stationary,
        # hs moving in float32r at 1 col/cycle)
        ps_u = ps_big.tile([1, H], F32, tag="big")
        for half in range(2):
            for c in range(HTC):
                i = half * HTC + c
                for nn in range(2):
                    nc.tensor.matmul(
                        ps_u[:, nn * 512 : (nn + 1) * 512],
                        lhsT=w_sb[:, i : i + 1],
                        rhs=halves[half][:, c, nn * 512 : (nn + 1) * 512],
                        start=(i == 0),
                        stop=(i == TC - 1),
                    )
        # normalize while copying out of PSUM: ctx_row = U / denom
        ctx_row = sb_batch.tile([1, H], F32, tag="ctxrow", bufs=1)
        nc.scalar.activation(out=ctx_row, in_=ps_u, func=ACTF.Copy, scale=inv_b)
        # transpose into ctxT[:, :, b] (single PSUM tile, one copy on ScalarE
        # to keep DVE free for the score stream)
        ps_ct = ps_tr.tile([P, HC, 1], F32, tag="tr")
        for c in range(HC):
            nc.tensor.transpose(
                ps_ct[:, c, :], ctx_row[0:1, c * P : (c + 1) * P], identity[0:1, 0:1]
            )
        nc.scalar.copy(out=ctxT[:, :, b : b + 1], in_=ps_ct)

    gates = {}
    prev = None
    for b in range(BL):
        halves = emit_load(b, gate=gates.get(b))
        score_sb = emit_score(b, halves)
        if prev is not None:
            emit_ctx(*prev)
        sm = emit_softmax(b, halves, score_sb)
        prev = (b, halves, sm[0], sm[1])
    emit_ctx(*prev)

    # ---------------- final projection (ctx half; h_t half ran in phase 0) --
    for c in range(HC):
        nc.tensor.matmul(
            ps_ab, lhsT=ctxT[:, c, :], rhs=wv_ctx[:, c, :],
            start=False, stop=(c == HC - 1),
        )

    res = small.tile([BL, OUT], F32)
    nc.vector.tensor_add(res, ps_ab, bv_b4)
    out_sb = small.tile([BL, OUT], F32)
    nc.scalar.activation(out=out_sb, in_=res, func=ACTF.Tanh)
    nc.sync.dma_start(out=y, in_=out_sb)


def build_program():
    nc = bacc.Bacc("TRN2", target_bir_lowering=False, debug=False)
    hs = nc.dram_tensor("hs", [BL, T, H], F32, kind="ExternalInput").ap()
    ht_in = nc.dram_tensor("ht", [BL, H], F32, kind="ExternalInput").ap()
    v_in = nc.dram_tensor("v", [BL, H], F32, kind="ExternalInput").ap()
    wv = nc.dram_tensor("Wv", [2 * H, OUT], F32, kind="ExternalInput").ap()
    bv = nc.dram_tensor("bv", [OUT], F32, kind="ExternalInput").ap()
    cin = nc.dram_tensor("cin", [P, P], F32, kind="ExternalInput").ap()
    y = nc.dram_tensor("y", [BL, OUT], F32, kind="ExternalOutput").ap()

    from contextlib import ExitStack

    with tile.TileContext(nc) as tc:
        with ExitStack() as ctx:
            _build_body(ctx, nc, tc, hs, ht_in, v_in, wv, bv, cin, y)
    nc.compile()
    return nc


_NC_CACHE = None


def _get_program():
    global _NC_CACHE
    if _NC_CACHE is None:
        _NC_CACHE = build_program()
    return _NC_CACHE


def make_cin():
    return np.eye(P, dtype=np.float32)


def kernel(hidden_states, W1, b1, Wv, bv, _trace=False, _trace_kwargs=None):
    hs = np.ascontiguousarray(np.asarray(hidden_states, dtype=np.float32))
    W1 = np.ascontiguousarray(np.asarray(W1, dtype=np.float32))
    b1 = np.ascontiguousarray(np.asarray(b1, dtype=np.float32))
    Wv = np.ascontiguousarray(np.asarray(Wv, dtype=np.float32))
    bv = np.ascontiguousarray(np.asarray(bv, dtype=np.float32))
    ht32 = np.ascontiguousarray(hs[:, -1, :] @ W1 + b1)     # [B, H]
    v32 = np.ascontiguousarray(ht32 @ W1.T)                  # [B, H]

    nc = _get_program()
    cin = make_cin()
    in_maps = [
        {
            "hs": hs[i * BL : (i + 1) * BL],
            "ht": ht32[i * BL : (i + 1) * BL],
            "v": v32[i * BL : (i + 1) * BL],
            "Wv": Wv,
            "bv": bv,
            "cin": cin,
        }
        for i in range(NCORES)
    ]
    kwargs = {}
    if _trace:
        kwargs["trace"] = True
        kwargs.update(_trace_kwargs or {})
    res = run_bass_kernel_spmd(nc, in_maps, core_ids=list(range(NCORES)), **kwargs)
    out = np.concatenate([res.results[i]["y"] for i in range(NCORES)], axis=0)
    if _trace:
        return out, res
    return out


if __name__ == "__main__":
    rng = np.random.default_rng(0)
    ins = {
        "hidden_states": rng.standard_normal((B, T, H), dtype=np.float32),
        "W1": (rng.standard_normal((H, H), dtype=np.float32) / np.sqrt(H)).astype(
            np.float32
        ),
        "b1": np.zeros(H, np.float32),
        "Wv": (
            rng.standard_normal((2 * H, OUT), dtype=np.float32) / np.sqrt(2 * H)
        ).astype(np.float32),
        "bv": np.zeros(OUT, np.float32),
    }
    out = kernel(**ins)
    print("out", out.shape, out.dtype, float(np.abs(out).mean()))
